# revision 22
# baseline (speedup 1.0000x reference)
"""Trainium2 Bass kernel for nn_AittalaGCN1dBlock (3-layer GCN block stack).

Self-contained: kernel(**inputs) takes FULL inputs, returns FULL output.

Strategy
--------
- GCN message passing as a dense [2048 x 2048] adjacency matmul (gcn_norm
  folded in, built on host; avg degree ~17 but the PE beats sparse paths).
- Data-parallel over graphs: core k handles b = k//2, n in [8*(k%2), ...+8).
  Cross-node max over n: AllReduce(max) within core pairs {2b, 2b+1},
  chunked along l so the ARs overlap the A-phase. BN stats: one tiny
  AllReduce(add) over all 8 cores per block.
- All tensor data fp16 (fp8 fails accuracy: e4m3's ~3% relative error does
  NOT average out in the GCN sum - measured 8.5e-2 rel err vs 2e-2 budget).
- Collective channel facts (measured): first collective pays ~80us warmup
  -> dummy warmup ARs fired at t=0; pair-ARs from different pairs run
  concurrently; per-core ARs drain at ~10us each (8us + 2us gap).
- W-phase (blocks 2/3) is LDWEIGHTS-bound (stationary swaps every matmul,
  ~107ns vs 53ns stream): the amax half's product n(amax) @ W_hi is shared
  by all 8 graphs on a core -> computed once into hWx, added during the
  per-graph psum evac (DVE add replaces the copy: free). 144 loads vs 256.
- Per-block tail floor = last amax chunk AR + stats AR (serial by data
  dependence: hi-half BN stats need the pair-max amax). Glue minimized:
  both halves' affine computed in packed [128,2] ops, normalize split
  ACT/DVE ordered so the next block's first matmuls unblock earliest.
"""

import os
import numpy as np

B, N, L = 4, 16, 2048
C0 = 64
NCORES = 8
EPS = 1e-5
PAIRS = [[0, 1], [2, 3], [4, 5], [6, 7]]
ALLCORES = [list(range(NCORES))]

_CACHE = {}


def _build_A_T(edge_index):
    """A_T[src, dst]: out[:, dst] = sum_src hW[src, :] * A_T[src, dst].

    PyG gcn_norm with improved=True: self loops weight 2, symmetric norm.
    """
    src = np.asarray(edge_index[0], np.int64)
    dst = np.asarray(edge_index[1], np.int64)
    deg = np.zeros(L, np.float32)
    np.add.at(deg, dst, np.float32(1.0))
    deg += 2.0
    dinv = (1.0 / np.sqrt(deg)).astype(np.float32)
    A_T = np.zeros((L, L), np.float32)
    np.add.at(A_T, (src, dst), dinv[src] * dinv[dst])
    idx = np.arange(L)
    A_T[idx, idx] += 2.0 * dinv * dinv
    return A_T


def _build_nc():
    from contextlib import ExitStack
    from concourse import bass, mybir, tile, bacc

    dt = mybir.dt
    F16, F32 = dt.float16, dt.float32
    AF = mybir.ActivationFunctionType
    ALU = mybir.AluOpType

    nc = bacc.Bacc("TRN2", target_bir_lowering=False, debug=False,
                   num_devices=NCORES)

    x_in = nc.dram_tensor("x_pack", [128, 4, 16, 128], F16,
                          kind="ExternalInput").ap()
    a_in = nc.dram_tensor("a_t", [4, 128, 16, 512], F16,
                          kind="ExternalInput").ap()
    w1_in = nc.dram_tensor("w1p", [128, 2, 128], F16, kind="ExternalInput").ap()
    w2_in = nc.dram_tensor("w2c", [128, 2, 128], F16, kind="ExternalInput").ap()
    w3_in = nc.dram_tensor("w3c", [128, 2, 128], F16, kind="ExternalInput").ap()
    # par[k] columns: (b, b, g_lo, g_hi, be_lo, be_hi, 0, 0)
    par_in = nc.dram_tensor("par", [3, 128, 8], F32, kind="ExternalInput").ap()
    # fp16 outputs (host casts to fp32); the shared amax half written once.
    out_dram = nc.dram_tensor("out", [8, 128, L], F16, kind="ExternalOutput").ap()
    out2_dram = nc.dram_tensor("out2", [128, L], F16, kind="ExternalOutput").ap()

    # collective bounce buffers (DRAM only; SBUF collectives are banned).
    # The collective channel takes ~80us from kernel start to come up
    # (measured); one dummy AR absorbs that warmup off the critical path.
    # Block1 finishes compute before the channel is ready, so it uses a
    # single full-width amax AR; blocks 2/3 use 4 chunks that overlap the
    # A-phase.
    warm_in = nc.dram_tensor("warm_in", [128, 4], F32)
    warm_p_out = nc.dram_tensor("warm_p_out", [128, 4], F32)
    amax1_in_d = nc.dram_tensor("amax1_in", [128, 2048], F16)
    amax1_out_d = nc.dram_tensor("amax1_out", [128, 2048], F16)
    amax_in_d = [None] + [[nc.dram_tensor(f"amax_in{k}_{ch}", [128, 512], F16)
                           for ch in range(4)] for k in (1, 2)]
    amax_out_d = [None] + [[nc.dram_tensor(f"amax_out{k}_{ch}", [128, 512],
                                           F16) for ch in range(4)]
                           for k in (1, 2)]
    st_in_d = [nc.dram_tensor(f"st_in{k}", [128, 4], F32) for k in range(3)]
    st_out_d = [nc.dram_tensor(f"st_out{k}", [128, 4], F32,
                               addr_space="Shared") for k in range(3)]

    with tile.TileContext(nc) as tc, ExitStack() as ctx:
        const = ctx.enter_context(tc.tile_pool(name="const", bufs=1))
        psum_a = ctx.enter_context(tc.tile_pool(name="psum_a", bufs=4,
                                                space="PSUM"))
        psum_w = ctx.enter_context(tc.tile_pool(name="psum_w", bufs=4,
                                                space="PSUM"))
        hbufs = ctx.enter_context(tc.tile_pool(name="hbufs", bufs=1))
        work = ctx.enter_context(tc.tile_pool(name="work", bufs=2))

        # ---- collective-channel warmup: fire one dummy AR immediately ----
        wz = const.tile([128, 4], F32)
        nc.vector.memset(wz, 0.0)
        nc.sync.dma_start(warm_in.ap(), wz)
        nc.gpsimd.collective_compute(
            "AllReduce", ALU.max, replica_groups=PAIRS,
            ins=[warm_in.ap().opt()], outs=[warm_p_out.ap().opt()])

        # ---- resident constants, DMA'd in exact consumption order: the
        # aggregate input-DMA rate (~230 GB/s) is slower than the A-phase
        # consumes, so ordering is what keeps the PE fed ----
        A_sb = const.tile([128, 4, 16, 512], F16)
        x_sb = const.tile([128, 4, 16, 128], F16)
        nc.sync.dma_start(x_sb[:, 0, 0:8], x_in[:, 0, 0:8])
        nc.sync.dma_start(x_sb[:, 0, 8:16], x_in[:, 0, 8:16])
        for st in range(0, 16, 2):
            nc.sync.dma_start(A_sb[:, 0, st:st + 2], a_in[0, :, st:st + 2])
        for pk in range(1, 4):
            nc.sync.dma_start(x_sb[:, pk, 0:8], x_in[:, pk, 0:8])
            nc.sync.dma_start(x_sb[:, pk, 8:16], x_in[:, pk, 8:16])
        W1_sb = const.tile([128, 2, 128], F16)
        nc.sync.dma_start(W1_sb, w1_in)
        for ch in range(1, 4):
            for st in range(0, 16, 2):
                nc.sync.dma_start(A_sb[:, ch, st:st + 2],
                                  a_in[ch, :, st:st + 2])
        W2_sb = const.tile([128, 2, 128], F16)
        nc.sync.dma_start(W2_sb, w2_in)
        W3_sb = const.tile([128, 2, 128], F16)
        nc.sync.dma_start(W3_sb, w3_in)
        par_sb = const.tile([128, 3, 8], F32)
        nc.sync.dma_start(par_sb, par_in.rearrange("k p f -> p k f"))

        # ---- persistent working buffers ----
        # aA[g] serves as both block-k 'a' and block-k+1 'h' (normalize is
        # in place); Tile's WAR tracking orders the overwrites.
        aA = [hbufs.tile([128, L], F16, name=f"aA{g}") for g in range(8)]
        hW_all = [hbufs.tile([128, 16, 128], F16, name=f"hW{g}")
                  for g in range(8)]
        hWx = hbufs.tile([128, 16, 128], F16, name="hWx")
        amx_glob = [hbufs.tile([128, L], F16, name=f"amxg{k}") for k in range(3)]
        amx_loc = hbufs.tile([128, L], F16, name="amx_loc")
        stats_sb = hbufs.tile([128, 192], F32, name="stats_sb")
        astat_sb = hbufs.tile([128, 24], F32, name="astat_sb")

        def evac(ps, g, ch):
            """psum [128c, 512] -> a fp16 (ACT; sole psum reader, avoids
            psum port contention with the PE) + bn_stats and running amax
            on DVE from the fp16 copy (2x DVE rate, stats on fp16-rounded
            values are well within budget)."""
            sl = slice(ch * 512, (ch + 1) * 512)
            nc.scalar.activation(aA[g][:, sl], ps, AF.Copy)
            nc.vector.bn_stats(stats_sb[:, (g * 4 + ch) * 6:(g * 4 + ch + 1) * 6],
                               aA[g][:, sl])
            if g == 0:
                nc.vector.tensor_copy(amx_loc[:, sl], aA[g][:, sl])
            else:
                nc.vector.tensor_max(amx_loc[:, sl], amx_loc[:, sl],
                                     aA[g][:, sl])

        def fire_amax_chunk(k, ch):
            """Pair AllReduce(max) of one 512-wide amax chunk; overlaps
            compute."""
            sl = slice(ch * 512, (ch + 1) * 512)
            nc.sync.dma_start(amax_in_d[k][ch].ap(), amx_loc[:, sl])
            nc.gpsimd.collective_compute(
                "AllReduce", ALU.max, replica_groups=PAIRS,
                ins=[amax_in_d[k][ch].ap().opt()],
                outs=[amax_out_d[k][ch].ap().opt()])
            nc.sync.dma_start(amx_glob[k][:, sl], amax_out_d[k][ch].ap())
            nc.vector.bn_stats(astat_sb[:, ch * 6:(ch + 1) * 6],
                               amx_glob[k][:, sl])

        def prep_lo_stats(k):
            """Aggregate the lo half + write its payload columns; runs
            during the last amax chunk's AR."""
            loc1 = work.tile([128, 2], F32, name="loc1", tag=f"lo{k}")
            nc.vector.bn_aggr(loc1, stats_sb)
            pay = work.tile([128, 4], F32, name="pay", tag=f"pay{k}")
            tmp = work.tile([128, 1], F32, name="tmp", tag=f"tmp{k}")
            nc.vector.tensor_copy(pay[:, 0:1], loc1[:, 0:1])
            nc.vector.tensor_mul(tmp, loc1[:, 0:1], loc1[:, 0:1])
            nc.vector.tensor_add(pay[:, 2:3], tmp, loc1[:, 1:2])
            return pay, tmp

        def stats_ar_and_affine(k, pay, tmp, n_agroups):
            """Hi-half payload + combined stats AR -> packed (scale, shift).

            Returns (sc, sh) [128, 2] with col 0 = lo half, col 1 = hi."""
            loc2 = work.tile([128, 2], F32, name="loc2")
            nc.vector.bn_aggr(loc2, astat_sb[:, 0:6 * n_agroups])
            nc.vector.tensor_copy(pay[:, 1:2], loc2[:, 0:1])
            nc.vector.tensor_mul(tmp, loc2[:, 0:1], loc2[:, 0:1])
            nc.vector.tensor_add(pay[:, 3:4], tmp, loc2[:, 1:2])
            nc.sync.dma_start(st_in_d[k].ap(), pay)
            nc.gpsimd.collective_compute(
                "AllReduce", ALU.add, replica_groups=ALLCORES,
                ins=[st_in_d[k].ap().opt()], outs=[st_out_d[k].ap().opt()])
            gstat = work.tile([128, 4], F32, name="gstat")
            nc.sync.dma_start(gstat, st_out_d[k].ap())
            gm = work.tile([128, 4], F32, name="gm")
            nc.vector.tensor_scalar_mul(gm, gstat, 1.0 / NCORES)
            v = work.tile([128, 2], F32, name="v")
            nc.vector.tensor_mul(v, gm[:, 0:2], gm[:, 0:2])
            nc.vector.tensor_sub(v, gm[:, 2:4], v)
            nc.vector.tensor_scalar_add(v, v, EPS)
            r = work.tile([128, 2], F32, name="r")
            nc.vector.reciprocal(r, v)
            s = work.tile([128, 2], F32, name="s")
            nc.scalar.sqrt(s, r)
            sc = work.tile([128, 2], F32, name="sc", tag=f"sc{k}")
            nc.vector.tensor_mul(sc, s, par_sb[:, k, 2:4])
            me = work.tile([128, 2], F32, name="me")
            nc.vector.tensor_add(me, gm[:, 0:2], par_sb[:, k, 0:2])
            nc.vector.tensor_mul(me, me, sc)
            sh = work.tile([128, 2], F32, name="sh", tag=f"sh{k}")
            nc.vector.tensor_sub(sh, par_sb[:, k, 4:6], me)
            return sc, sh

        def normalize_inplace(k, sc, sh):
            """relu-affine both halves; ordered so block k+1's first
            matmuls (hWx: needs amx; per-g: needs aA[g]) unblock fastest.
            amx on ACT (ACT also evacuates the W-phase psums next); aA on
            DVE and GPSIMD in g order."""
            nc.scalar.activation(amx_glob[k], amx_glob[k], AF.Relu,
                                 bias=sh[:, 1:2], scale=sc[:, 1:2])
            for g in range(8):
                eng = nc.vector if g % 2 == 0 else nc.gpsimd
                eng.tensor_scalar(aA[g], aA[g], sc[:, 0:1], sh[:, 0:1],
                                  ALU.mult, ALU.add)
                eng.tensor_scalar_max(aA[g], aA[g], 0.0)

        # ================= block 1 =================
        # compute ends before the collective channel is up (~80us), so a
        # single full-width amax AR beats 4 chunks: fewer serial drains.
        with tc.tile_pool(name="blk1", bufs=1) as blk1:
            Ah_sb = blk1.tile([128, 4, 2048], F16)
            for ch in range(4):
                csl = slice(ch * 512, (ch + 1) * 512)
                for pk in range(4):
                    ps = psum_a.tile([128, 512], F32, name="ps_a", tag="ps_a")
                    for st in range(16):
                        nc.tensor.matmul(ps, lhsT=x_sb[:, pk, st, :],
                                         rhs=A_sb[:, ch, st, :],
                                         start=(st == 0), stop=(st == 15))
                    nc.scalar.activation(Ah_sb[:, pk, csl], ps, AF.Copy)
                for pk in range(4):
                    for j in range(2):
                        g = 2 * pk + j
                        ps2 = psum_a.tile([128, 512], F32, name="ps_w1",
                                          tag="ps_a")
                        nc.tensor.matmul(ps2, lhsT=W1_sb[:, j, :],
                                         rhs=Ah_sb[:, pk, csl],
                                         start=True, stop=True)
                        evac(ps2, g, ch)
            nc.sync.dma_start(amax1_in_d.ap(), amx_loc)
            nc.gpsimd.collective_compute(
                "AllReduce", ALU.max, replica_groups=PAIRS,
                ins=[amax1_in_d.ap().opt()], outs=[amax1_out_d.ap().opt()])
            pay0, tmp0 = prep_lo_stats(0)
            nc.sync.dma_start(amx_glob[0], amax1_out_d.ap())
            for ch in range(4):
                nc.vector.bn_stats(astat_sb[:, ch * 6:(ch + 1) * 6],
                                   amx_glob[0][:, ch * 512:(ch + 1) * 512])
            sc, sh = stats_ar_and_affine(0, pay0, tmp0, 4)
            normalize_inplace(0, sc, sh)

        # ================= blocks 2 & 3 =================
        for k, W_sb in enumerate([W2_sb, W3_sb], start=1):
            h_amax = amx_glob[k - 1]
            # shared amax contribution: hWx = n(amax) @ W_hi, once per core
            for lt4 in range(4):
                psx = psum_w.tile([128, 4, 128], F32, name="ps_x", tag="ps_w")
                for q in range(4):
                    lt = lt4 * 4 + q
                    sl = slice(lt * 128, (lt + 1) * 128)
                    nc.tensor.matmul(psx[:, q, :], lhsT=h_amax[:, sl],
                                     rhs=W_sb[:, 1, :], start=True, stop=True)
                nc.scalar.activation(hWx[:, lt4 * 4:(lt4 + 1) * 4, :], psx,
                                     AF.Copy)
            for g in range(8):
                for lt4 in range(4):
                    q4 = slice(lt4 * 4, (lt4 + 1) * 4)
                    psw = psum_w.tile([128, 4, 128], F32, name="ps_w",
                                      tag="ps_w")
                    for q in range(4):
                        lt = lt4 * 4 + q
                        sl = slice(lt * 128, (lt + 1) * 128)
                        nc.tensor.matmul(psw[:, q, :], lhsT=aA[g][:, sl],
                                         rhs=W_sb[:, 0, :],
                                         start=True, stop=True)
                    # evac: psum copy (ACT mostly, DVE for two graphs to
                    # balance), then the shared amax term is added in
                    # place by the otherwise-idle GPSIMD (fp16, SBUF-only)
                    if g < 6:
                        nc.scalar.activation(hW_all[g][:, q4, :], psw,
                                             AF.Copy)
                    else:
                        nc.vector.tensor_copy(hW_all[g][:, q4, :], psw)
                    nc.gpsimd.tensor_add(hW_all[g][:, q4, :],
                                         hW_all[g][:, q4, :],
                                         hWx[:, q4, :])
            for ch in range(4):
                for g in range(8):
                    ps = psum_a.tile([128, 512], F32, name="ps_a2", tag="ps_a")
                    for st in range(16):
                        nc.tensor.matmul(ps, lhsT=hW_all[g][:, st, :],
                                         rhs=A_sb[:, ch, st, :],
                                         start=(st == 0), stop=(st == 15))
                    evac(ps, g, ch)
                if ch == 3:
                    pay_k, tmp_k = prep_lo_stats(k)
                fire_amax_chunk(k, ch)
            sc, sh = stats_ar_and_affine(k, pay_k, tmp_k, 4)
            if k == 1:
                normalize_inplace(1, sc, sh)
            else:
                # final: normalize into staging chunks, DMA out per chunk,
                # spread across ACT / DVE / GPSIMD so the 40 chunks drain
                # in ~3 engine-parallel waves.
                with tc.tile_pool(name="stage", bufs=12) as stage:
                    for ch in range(4):
                        sl = slice(ch * 512, (ch + 1) * 512)
                        s2 = stage.tile([128, 512], F16, name="stg2",
                                        tag="stg")
                        nc.scalar.activation(s2, amx_glob[2][:, sl], AF.Relu,
                                             bias=sh[:, 1:2], scale=sc[:, 1:2])
                        nc.sync.dma_start(out2_dram[:, sl], s2)
                    for ch in range(4):
                        sl = slice(ch * 512, (ch + 1) * 512)
                        for g in range(8):
                            sg = stage.tile([128, 512], F16, name="stg",
                                            tag="stg")
                            if g in (1, 4, 7):
                                nc.scalar.activation(sg, aA[g][:, sl], AF.Relu,
                                                     bias=sh[:, 0:1],
                                                     scale=sc[:, 0:1])
                            else:
                                eng = nc.vector if g % 2 == 0 else nc.gpsimd
                                eng.tensor_scalar(sg, aA[g][:, sl],
                                                  sc[:, 0:1], sh[:, 0:1],
                                                  ALU.mult, ALU.add)
                                eng.tensor_scalar_max(sg, sg, 0.0)
                            nc.sync.dma_start(out_dram[g, :, sl], sg)

    nc.compile()
    return nc


def _host_prep(x, edge_index, W1, b1, W2, b2, W3, b3,
               g1, be1, g2, be2, g3, be3):
    A_T = _build_A_T(edge_index).astype(np.float16)
    # [ch, p, st, j] = A_T[st*128+p, ch*512+j]
    a_t = np.ascontiguousarray(
        A_T.reshape(16, 128, 4, 512).transpose(2, 1, 0, 3))

    w1p = np.zeros([128, 2, 128], np.float16)
    w1p[0:64, 0, :] = W1
    w1p[64:128, 1, :] = W1
    w2c = np.ascontiguousarray(
        W2.astype(np.float16).reshape(2, 128, 128).transpose(1, 0, 2))
    w3c = np.ascontiguousarray(
        W3.astype(np.float16).reshape(2, 128, 128).transpose(1, 0, 2))

    par = np.zeros([3, 128, 8], np.float32)
    for k, (b_, g_, be_) in enumerate(
            [(b1, g1, be1), (b2, g2, be2), (b3, g3, be3)]):
        par[k, :, 0] = b_
        par[k, :, 1] = b_
        par[k, :, 2] = g_[:128]
        par[k, :, 3] = g_[128:]
        par[k, :, 4] = be_[:128]
        par[k, :, 5] = be_[128:]

    in_maps = []
    for core in range(NCORES):
        b_idx, nh = core // 2, core % 2
        xnm = np.ascontiguousarray(
            x[b_idx, nh * 8:nh * 8 + 8].transpose(0, 2, 1)).astype(np.float16)
        t = xnm.reshape(8, 16, 128, 64)  # [g, st, p, c]
        xp = np.zeros([128, 4, 16, 128], np.float16)
        for pk in range(4):
            xp[:, pk, :, 0:64] = t[2 * pk].transpose(1, 0, 2)
            xp[:, pk, :, 64:128] = t[2 * pk + 1].transpose(1, 0, 2)
        in_maps.append(dict(x_pack=xp, a_t=a_t, w1p=w1p, w2c=w2c, w3c=w3c,
                            par=par))
    return in_maps


def _get_nc():
    if "nc" not in _CACHE:
        _CACHE["nc"] = _build_nc()
    return _CACHE["nc"]


def _install_profiling_shim():
    """This image's antenv lacks axon_hooks; recreate it so trace=True works."""
    import sys
    import types
    if "antenv.axon_hooks" in sys.modules:
        return
    mod = types.ModuleType("antenv.axon_hooks")
    state = {"hook": None}
    mod.set_axon_ntff_profile_hook = lambda h: state.__setitem__("hook", h)
    mod.get_axon_ntff_profile_hook = lambda: state["hook"]
    sys.modules["antenv.axon_hooks"] = mod
    try:
        from trn_agent_boot.trn_boot import _ntff_profile_via_ctypes
        mod.set_axon_ntff_profile_hook(
            _ntff_profile_via_ctypes("/opt/axon/libaxon_pjrt.so"))
    except Exception:
        pass
    # zero-egress container: skip the artifact bucket upload
    import concourse.bass_utils as bu
    bu.upload_artifacts = lambda tmpdir: tmpdir


def _run(in_maps, trace=False):
    nc = _get_nc()
    kwargs = {}
    if trace:
        _install_profiling_shim()
        os.environ["BASS_PERFETTO_PROFILE_ALL_CORES"] = "1"
        kwargs["trace"] = True
    from concourse.bass_utils import run_bass_kernel_spmd
    res = run_bass_kernel_spmd(nc, in_maps, core_ids=list(range(NCORES)),
                               **kwargs)
    return res


def _assemble(results):
    out = np.zeros((B, N, 256, L), np.float32)
    for core in range(NCORES):
        b_idx, nh = core // 2, core % 2
        sl = slice(nh * 8, nh * 8 + 8)
        out[b_idx, sl, 0:128] = np.asarray(
            results[core]["out"]).reshape(8, 128, L).astype(np.float32)
        out[b_idx, sl, 128:256] = np.asarray(
            results[core]["out2"]).reshape(128, L).astype(np.float32)[None]
    return out.reshape(B * N, 256, L)


def kernel(**inputs):
    np_inputs = {k: np.asarray(v) for k, v in inputs.items()}
    in_maps = _host_prep(**{k: (np_inputs[k].astype(np.float32)
                                if k != "edge_index" else np_inputs[k])
                            for k in np_inputs})
    res = _run(in_maps, trace=False)
    return _assemble(res.results)


def kernel_traced(**inputs):
    """Returns (output, BassKernelResults) with NTFF profiling if available."""
    np_inputs = {k: np.asarray(v) for k, v in inputs.items()}
    in_maps = _host_prep(**{k: (np_inputs[k].astype(np.float32)
                                if k != "edge_index" else np_inputs[k])
                            for k in np_inputs})
    res = _run(in_maps, trace=True)
    return _assemble(res.results), res


# revision 23
# speedup vs baseline: 1.5411x; 1.5411x over previous
"""Trainium2 Bass kernel for nn_AittalaGCN1dBlock (3-layer GCN block stack).

Self-contained: kernel(**inputs) takes FULL inputs, returns FULL output.

Strategy
--------
- GCN message passing as a dense [2048 x 2048] adjacency matmul (gcn_norm
  folded in, built on host; avg degree ~17 but the PE beats sparse paths).
- Data-parallel over graphs: core k handles b = k//2, n in [8*(k%2), ...+8).
  Cross-node max over n: AllReduce(max) within core pairs {2b, 2b+1},
  chunked along l so the ARs overlap the A-phase. BN stats: one tiny
  AllReduce(add) over all 8 cores per block.
- All tensor data fp16 (fp8 fails accuracy: e4m3's ~3% relative error does
  NOT average out in the GCN sum - measured 8.5e-2 rel err vs 2e-2 budget).
- Collective channel facts (measured): first collective pays ~80us warmup
  -> dummy warmup ARs fired at t=0; pair-ARs from different pairs run
  concurrently; per-core ARs drain at ~10us each (8us + 2us gap).
- W-phase (blocks 2/3) is LDWEIGHTS-bound (stationary swaps every matmul,
  ~107ns vs 53ns stream): the amax half's product n(amax) @ W_hi is shared
  by all 8 graphs on a core -> computed once into hWx, added during the
  per-graph psum evac (DVE add replaces the copy: free). 144 loads vs 256.
- Per-block tail floor = last amax chunk AR + stats AR (serial by data
  dependence: hi-half BN stats need the pair-max amax). Glue minimized:
  both halves' affine computed in packed [128,2] ops, normalize split
  ACT/DVE ordered so the next block's first matmuls unblock earliest.
"""

import os
import numpy as np

B, N, L = 4, 16, 2048
C0 = 64
NCORES = 8
EPS = 1e-5
PAIRS = [[0, 1], [2, 3], [4, 5], [6, 7]]
ALLCORES = [list(range(NCORES))]

_CACHE = {}


def _build_A_T(edge_index):
    """A_T[src, dst]: out[:, dst] = sum_src hW[src, :] * A_T[src, dst].

    PyG gcn_norm with improved=True: self loops weight 2, symmetric norm.
    """
    src = np.asarray(edge_index[0], np.int64)
    dst = np.asarray(edge_index[1], np.int64)
    deg = np.zeros(L, np.float32)
    np.add.at(deg, dst, np.float32(1.0))
    deg += 2.0
    dinv = (1.0 / np.sqrt(deg)).astype(np.float32)
    A_T = np.zeros((L, L), np.float32)
    np.add.at(A_T, (src, dst), dinv[src] * dinv[dst])
    idx = np.arange(L)
    A_T[idx, idx] += 2.0 * dinv * dinv
    return A_T


def _build_nc():
    from contextlib import ExitStack
    from concourse import bass, mybir, tile, bacc

    dt = mybir.dt
    F16, F32 = dt.float16, dt.float32
    AF = mybir.ActivationFunctionType
    ALU = mybir.AluOpType

    nc = bacc.Bacc("TRN2", target_bir_lowering=False, debug=False,
                   num_devices=NCORES)

    x_in = nc.dram_tensor("x_pack", [128, 4, 16, 128], F16,
                          kind="ExternalInput").ap()
    a_in = nc.dram_tensor("a_t", [4, 128, 16, 512], F16,
                          kind="ExternalInput").ap()
    w1_in = nc.dram_tensor("w1p", [128, 2, 128], F16, kind="ExternalInput").ap()
    w2_in = nc.dram_tensor("w2c", [128, 2, 128], F16, kind="ExternalInput").ap()
    w3_in = nc.dram_tensor("w3c", [128, 2, 128], F16, kind="ExternalInput").ap()
    # par[k] columns: (b, b, g_lo, g_hi, be_lo, be_hi, 0, 0)
    par_in = nc.dram_tensor("par", [3, 128, 8], F32, kind="ExternalInput").ap()
    # fp16 outputs (host casts to fp32); the shared amax half written once.
    out_dram = nc.dram_tensor("out", [8, 128, L], F16, kind="ExternalOutput").ap()
    out2_dram = nc.dram_tensor("out2", [128, L], F16, kind="ExternalOutput").ap()

    # collective bounce buffers (DRAM only; SBUF collectives are banned).
    # The collective channel takes ~80us from kernel start to come up
    # (measured); one dummy AR absorbs that warmup off the critical path.
    # Block1 finishes compute before the channel is ready, so it uses a
    # single full-width amax AR; blocks 2/3 use 4 chunks that overlap the
    # A-phase.
    warm_in = nc.dram_tensor("warm_in", [128, 4], F32)
    warm_p_out = nc.dram_tensor("warm_p_out", [128, 4], F32)
    amax1_in_d = nc.dram_tensor("amax1_in", [128, 2048], F16)
    amax1_out_d = nc.dram_tensor("amax1_out", [128, 2048], F16)
    amax_in_d = [None] + [[nc.dram_tensor(f"amax_in{k}_{ch}", [128, 512], F16)
                           for ch in range(4)] for k in (1, 2)]
    amax_out_d = [None] + [[nc.dram_tensor(f"amax_out{k}_{ch}", [128, 512],
                                           F16) for ch in range(4)]
                           for k in (1, 2)]
    st_in_d = [nc.dram_tensor(f"st_in{k}", [128, 4], F32) for k in range(3)]
    st_out_d = [nc.dram_tensor(f"st_out{k}", [128, 4], F32,
                               addr_space="Shared") for k in range(3)]

    with tile.TileContext(nc) as tc, ExitStack() as ctx:
        const = ctx.enter_context(tc.tile_pool(name="const", bufs=1))
        psum_a = ctx.enter_context(tc.tile_pool(name="psum_a", bufs=4,
                                                space="PSUM"))
        psum_w = ctx.enter_context(tc.tile_pool(name="psum_w", bufs=4,
                                                space="PSUM"))
        hbufs = ctx.enter_context(tc.tile_pool(name="hbufs", bufs=1))
        work = ctx.enter_context(tc.tile_pool(name="work", bufs=2))

        # ---- collective-channel warmup: fire one dummy AR immediately ----
        wz = const.tile([128, 4], F32)
        nc.vector.memset(wz, 0.0)
        nc.sync.dma_start(warm_in.ap(), wz)
        nc.gpsimd.collective_compute(
            "AllReduce", ALU.max, replica_groups=PAIRS,
            ins=[warm_in.ap().opt()], outs=[warm_p_out.ap().opt()])

        # ---- resident constants, DMA'd in exact consumption order: the
        # aggregate input-DMA rate (~230 GB/s) is slower than the A-phase
        # consumes, so ordering is what keeps the PE fed ----
        A_sb = const.tile([128, 4, 16, 512], F16)
        x_sb = const.tile([128, 4, 16, 128], F16)
        nc.sync.dma_start(x_sb[:, 0, 0:8], x_in[:, 0, 0:8])
        nc.sync.dma_start(x_sb[:, 0, 8:16], x_in[:, 0, 8:16])
        for st in range(0, 16, 2):
            nc.sync.dma_start(A_sb[:, 0, st:st + 2], a_in[0, :, st:st + 2])
        for pk in range(1, 4):
            nc.sync.dma_start(x_sb[:, pk, 0:8], x_in[:, pk, 0:8])
            nc.sync.dma_start(x_sb[:, pk, 8:16], x_in[:, pk, 8:16])
        W1_sb = const.tile([128, 2, 128], F16)
        nc.sync.dma_start(W1_sb, w1_in)
        for ch in range(1, 4):
            for st in range(0, 16, 2):
                nc.sync.dma_start(A_sb[:, ch, st:st + 2],
                                  a_in[ch, :, st:st + 2])
        W2_sb = const.tile([128, 2, 128], F16)
        nc.sync.dma_start(W2_sb, w2_in)
        W3_sb = const.tile([128, 2, 128], F16)
        nc.sync.dma_start(W3_sb, w3_in)
        par_sb = const.tile([128, 3, 8], F32)
        nc.sync.dma_start(par_sb, par_in.rearrange("k p f -> p k f"))

        # ---- persistent working buffers ----
        # aA[g] serves as both block-k 'a' and block-k+1 'h' (normalize is
        # in place); Tile's WAR tracking orders the overwrites.
        aA = [hbufs.tile([128, L], F16, name=f"aA{g}") for g in range(8)]
        hW_all = [hbufs.tile([128, 16, 128], F16, name=f"hW{g}")
                  for g in range(8)]
        hWx = hbufs.tile([128, 16, 128], F16, name="hWx")
        amx_glob = [hbufs.tile([128, L], F16, name=f"amxg{k}") for k in range(3)]
        amx_loc = hbufs.tile([128, L], F16, name="amx_loc")
        stats_sb = hbufs.tile([128, 192], F32, name="stats_sb")
        astat_sb = hbufs.tile([128, 24], F32, name="astat_sb")

        def evac(ps, g, ch):
            """psum [128c, 512] -> a fp16 (ACT; sole psum reader, avoids
            psum port contention with the PE) + bn_stats and running amax
            on DVE from the fp16 copy (2x DVE rate, stats on fp16-rounded
            values are well within budget)."""
            sl = slice(ch * 512, (ch + 1) * 512)
            nc.scalar.activation(aA[g][:, sl], ps, AF.Copy)
            nc.vector.bn_stats(stats_sb[:, (g * 4 + ch) * 6:(g * 4 + ch + 1) * 6],
                               ps)
            if g == 0:
                nc.vector.tensor_copy(amx_loc[:, sl], aA[g][:, sl])
            else:
                nc.vector.tensor_max(amx_loc[:, sl], amx_loc[:, sl],
                                     aA[g][:, sl])

        def fire_amax_chunk(k, ch):
            """Pair AllReduce(max) of one 512-wide amax chunk; overlaps
            compute."""
            sl = slice(ch * 512, (ch + 1) * 512)
            nc.sync.dma_start(amax_in_d[k][ch].ap(), amx_loc[:, sl])
            nc.gpsimd.collective_compute(
                "AllReduce", ALU.max, replica_groups=PAIRS,
                ins=[amax_in_d[k][ch].ap().opt()],
                outs=[amax_out_d[k][ch].ap().opt()])
            nc.sync.dma_start(amx_glob[k][:, sl], amax_out_d[k][ch].ap())
            nc.vector.bn_stats(astat_sb[:, ch * 6:(ch + 1) * 6],
                               amx_glob[k][:, sl])

        def prep_lo_stats(k):
            """Aggregate the lo half + write its payload columns; runs
            during the last amax chunk's AR."""
            loc1 = work.tile([128, 2], F32, name="loc1", tag=f"lo{k}")
            nc.vector.bn_aggr(loc1, stats_sb)
            pay = work.tile([128, 4], F32, name="pay", tag=f"pay{k}")
            tmp = work.tile([128, 1], F32, name="tmp", tag=f"tmp{k}")
            nc.vector.tensor_copy(pay[:, 0:1], loc1[:, 0:1])
            nc.vector.tensor_mul(tmp, loc1[:, 0:1], loc1[:, 0:1])
            nc.vector.tensor_add(pay[:, 2:3], tmp, loc1[:, 1:2])
            return pay, tmp

        def stats_ar_and_affine(k, pay, tmp, n_agroups):
            """Hi-half payload + combined stats AR -> packed (scale, shift).

            Returns (sc, sh) [128, 2] with col 0 = lo half, col 1 = hi."""
            loc2 = work.tile([128, 2], F32, name="loc2")
            nc.vector.bn_aggr(loc2, astat_sb[:, 0:6 * n_agroups])
            nc.vector.tensor_copy(pay[:, 1:2], loc2[:, 0:1])
            nc.vector.tensor_mul(tmp, loc2[:, 0:1], loc2[:, 0:1])
            nc.vector.tensor_add(pay[:, 3:4], tmp, loc2[:, 1:2])
            nc.sync.dma_start(st_in_d[k].ap(), pay)
            nc.gpsimd.collective_compute(
                "AllReduce", ALU.add, replica_groups=ALLCORES,
                ins=[st_in_d[k].ap().opt()], outs=[st_out_d[k].ap().opt()])
            gstat = work.tile([128, 4], F32, name="gstat")
            nc.sync.dma_start(gstat, st_out_d[k].ap())
            gm = work.tile([128, 4], F32, name="gm")
            nc.vector.tensor_scalar_mul(gm, gstat, 1.0 / NCORES)
            v = work.tile([128, 2], F32, name="v")
            nc.vector.tensor_mul(v, gm[:, 0:2], gm[:, 0:2])
            nc.vector.tensor_sub(v, gm[:, 2:4], v)
            nc.vector.tensor_scalar_add(v, v, EPS)
            r = work.tile([128, 2], F32, name="r")
            nc.vector.reciprocal(r, v)
            s = work.tile([128, 2], F32, name="s")
            nc.scalar.sqrt(s, r)
            sc = work.tile([128, 2], F32, name="sc", tag=f"sc{k}")
            nc.vector.tensor_mul(sc, s, par_sb[:, k, 2:4])
            me = work.tile([128, 2], F32, name="me")
            nc.vector.tensor_add(me, gm[:, 0:2], par_sb[:, k, 0:2])
            nc.vector.tensor_mul(me, me, sc)
            sh = work.tile([128, 2], F32, name="sh", tag=f"sh{k}")
            nc.vector.tensor_sub(sh, par_sb[:, k, 4:6], me)
            return sc, sh

        def normalize_inplace(k, sc, sh):
            """relu-affine both halves; ordered so block k+1's first
            matmuls (hWx: needs amx; per-g: needs aA[g]) unblock fastest.
            amx on ACT (ACT also evacuates the W-phase psums next); aA on
            DVE and GPSIMD in g order."""
            nc.scalar.activation(amx_glob[k], amx_glob[k], AF.Relu,
                                 bias=sh[:, 1:2], scale=sc[:, 1:2])
            for g in (0, 1, 2, 3):
                nc.vector.tensor_scalar(aA[g], aA[g], sc[:, 0:1], sh[:, 0:1],
                                        ALU.mult, ALU.add)
                nc.vector.tensor_scalar_max(aA[g], aA[g], 0.0)
            for g in (4, 5, 6, 7):
                nc.scalar.activation(aA[g], aA[g], AF.Relu,
                                     bias=sh[:, 0:1], scale=sc[:, 0:1])

        # ================= block 1 =================
        # compute ends before the collective channel is up (~80us), so a
        # single full-width amax AR beats 4 chunks: fewer serial drains.
        with tc.tile_pool(name="blk1", bufs=1) as blk1:
            Ah_sb = blk1.tile([128, 4, 2048], F16)
            for ch in range(4):
                csl = slice(ch * 512, (ch + 1) * 512)
                for pk in range(4):
                    ps = psum_a.tile([128, 512], F32, name="ps_a", tag="ps_a")
                    for st in range(16):
                        nc.tensor.matmul(ps, lhsT=x_sb[:, pk, st, :],
                                         rhs=A_sb[:, ch, st, :],
                                         start=(st == 0), stop=(st == 15))
                    nc.scalar.activation(Ah_sb[:, pk, csl], ps, AF.Copy)
                for pk in range(4):
                    for j in range(2):
                        g = 2 * pk + j
                        ps2 = psum_a.tile([128, 512], F32, name="ps_w1",
                                          tag="ps_a")
                        nc.tensor.matmul(ps2, lhsT=W1_sb[:, j, :],
                                         rhs=Ah_sb[:, pk, csl],
                                         start=True, stop=True)
                        evac(ps2, g, ch)
            nc.sync.dma_start(amax1_in_d.ap(), amx_loc)
            nc.gpsimd.collective_compute(
                "AllReduce", ALU.max, replica_groups=PAIRS,
                ins=[amax1_in_d.ap().opt()], outs=[amax1_out_d.ap().opt()])
            pay0, tmp0 = prep_lo_stats(0)
            nc.sync.dma_start(amx_glob[0], amax1_out_d.ap())
            for ch in range(4):
                nc.vector.bn_stats(astat_sb[:, ch * 6:(ch + 1) * 6],
                                   amx_glob[0][:, ch * 512:(ch + 1) * 512])
            sc, sh = stats_ar_and_affine(0, pay0, tmp0, 4)
            normalize_inplace(0, sc, sh)

        # ================= blocks 2 & 3 =================
        for k, W_sb in enumerate([W2_sb, W3_sb], start=1):
            h_amax = amx_glob[k - 1]
            # shared amax contribution: hWx = n(amax) @ W_hi, once per core
            for lt4 in range(4):
                psx = psum_w.tile([128, 4, 128], F32, name="ps_x", tag="ps_w")
                for q in range(4):
                    lt = lt4 * 4 + q
                    sl = slice(lt * 128, (lt + 1) * 128)
                    nc.tensor.matmul(psx[:, q, :], lhsT=h_amax[:, sl],
                                     rhs=W_sb[:, 1, :], start=True, stop=True)
                nc.scalar.activation(hWx[:, lt4 * 4:(lt4 + 1) * 4, :], psx,
                                     AF.Copy)
            for g in range(8):
                for lt4 in range(4):
                    q4 = slice(lt4 * 4, (lt4 + 1) * 4)
                    psw = psum_w.tile([128, 4, 128], F32, name="ps_w",
                                      tag="ps_w")
                    for q in range(4):
                        lt = lt4 * 4 + q
                        sl = slice(lt * 128, (lt + 1) * 128)
                        nc.tensor.matmul(psw[:, q, :], lhsT=aA[g][:, sl],
                                         rhs=W_sb[:, 0, :],
                                         start=True, stop=True)
                    # evac: DVE add fuses the shared amax term into the
                    # psum drain (GPSIMD tensor ops measured ~8x slower
                    # than DVE - a software DSP - never use them for bulk)
                    nc.vector.tensor_add(hW_all[g][:, q4, :], psw,
                                         hWx[:, q4, :])
            for ch in range(4):
                for g in range(8):
                    ps = psum_a.tile([128, 512], F32, name="ps_a2", tag="ps_a")
                    for st in range(16):
                        nc.tensor.matmul(ps, lhsT=hW_all[g][:, st, :],
                                         rhs=A_sb[:, ch, st, :],
                                         start=(st == 0), stop=(st == 15))
                    evac(ps, g, ch)
                if ch == 3:
                    pay_k, tmp_k = prep_lo_stats(k)
                fire_amax_chunk(k, ch)
            sc, sh = stats_ar_and_affine(k, pay_k, tmp_k, 4)
            if k == 1:
                normalize_inplace(1, sc, sh)
            else:
                # final: normalize into staging chunks, DMA out per chunk,
                # spread across ACT / DVE / GPSIMD so the 40 chunks drain
                # in ~3 engine-parallel waves.
                with tc.tile_pool(name="stage", bufs=12) as stage:
                    for ch in range(4):
                        sl = slice(ch * 512, (ch + 1) * 512)
                        s2 = stage.tile([128, 512], F16, name="stg2",
                                        tag="stg")
                        nc.scalar.activation(s2, amx_glob[2][:, sl], AF.Relu,
                                             bias=sh[:, 1:2], scale=sc[:, 1:2])
                        nc.sync.dma_start(out2_dram[:, sl], s2)
                    for ch in range(4):
                        sl = slice(ch * 512, (ch + 1) * 512)
                        for g in range(8):
                            sg = stage.tile([128, 512], F16, name="stg",
                                            tag="stg")
                            if g in (1, 4, 7):
                                nc.scalar.activation(sg, aA[g][:, sl], AF.Relu,
                                                     bias=sh[:, 0:1],
                                                     scale=sc[:, 0:1])
                            else:
                                nc.vector.tensor_scalar(sg, aA[g][:, sl],
                                                        sc[:, 0:1], sh[:, 0:1],
                                                        ALU.mult, ALU.add)
                                nc.vector.tensor_scalar_max(sg, sg, 0.0)
                            nc.sync.dma_start(out_dram[g, :, sl], sg)

    nc.compile()
    return nc


def _host_prep(x, edge_index, W1, b1, W2, b2, W3, b3,
               g1, be1, g2, be2, g3, be3):
    A_T = _build_A_T(edge_index).astype(np.float16)
    # [ch, p, st, j] = A_T[st*128+p, ch*512+j]
    a_t = np.ascontiguousarray(
        A_T.reshape(16, 128, 4, 512).transpose(2, 1, 0, 3))

    w1p = np.zeros([128, 2, 128], np.float16)
    w1p[0:64, 0, :] = W1
    w1p[64:128, 1, :] = W1
    w2c = np.ascontiguousarray(
        W2.astype(np.float16).reshape(2, 128, 128).transpose(1, 0, 2))
    w3c = np.ascontiguousarray(
        W3.astype(np.float16).reshape(2, 128, 128).transpose(1, 0, 2))

    par = np.zeros([3, 128, 8], np.float32)
    for k, (b_, g_, be_) in enumerate(
            [(b1, g1, be1), (b2, g2, be2), (b3, g3, be3)]):
        par[k, :, 0] = b_
        par[k, :, 1] = b_
        par[k, :, 2] = g_[:128]
        par[k, :, 3] = g_[128:]
        par[k, :, 4] = be_[:128]
        par[k, :, 5] = be_[128:]

    in_maps = []
    for core in range(NCORES):
        b_idx, nh = core // 2, core % 2
        xnm = np.ascontiguousarray(
            x[b_idx, nh * 8:nh * 8 + 8].transpose(0, 2, 1)).astype(np.float16)
        t = xnm.reshape(8, 16, 128, 64)  # [g, st, p, c]
        xp = np.zeros([128, 4, 16, 128], np.float16)
        for pk in range(4):
            xp[:, pk, :, 0:64] = t[2 * pk].transpose(1, 0, 2)
            xp[:, pk, :, 64:128] = t[2 * pk + 1].transpose(1, 0, 2)
        in_maps.append(dict(x_pack=xp, a_t=a_t, w1p=w1p, w2c=w2c, w3c=w3c,
                            par=par))
    return in_maps


def _get_nc():
    if "nc" not in _CACHE:
        _CACHE["nc"] = _build_nc()
    return _CACHE["nc"]


def _install_profiling_shim():
    """This image's antenv lacks axon_hooks; recreate it so trace=True works."""
    import sys
    import types
    if "antenv.axon_hooks" in sys.modules:
        return
    mod = types.ModuleType("antenv.axon_hooks")
    state = {"hook": None}
    mod.set_axon_ntff_profile_hook = lambda h: state.__setitem__("hook", h)
    mod.get_axon_ntff_profile_hook = lambda: state["hook"]
    sys.modules["antenv.axon_hooks"] = mod
    try:
        from trn_agent_boot.trn_boot import _ntff_profile_via_ctypes
        mod.set_axon_ntff_profile_hook(
            _ntff_profile_via_ctypes("/opt/axon/libaxon_pjrt.so"))
    except Exception:
        pass
    # zero-egress container: skip the artifact bucket upload
    import concourse.bass_utils as bu
    bu.upload_artifacts = lambda tmpdir: tmpdir


def _run(in_maps, trace=False):
    nc = _get_nc()
    kwargs = {}
    if trace:
        _install_profiling_shim()
        os.environ["BASS_PERFETTO_PROFILE_ALL_CORES"] = "1"
        kwargs["trace"] = True
    from concourse.bass_utils import run_bass_kernel_spmd
    res = run_bass_kernel_spmd(nc, in_maps, core_ids=list(range(NCORES)),
                               **kwargs)
    return res


def _assemble(results):
    out = np.zeros((B, N, 256, L), np.float32)
    for core in range(NCORES):
        b_idx, nh = core // 2, core % 2
        sl = slice(nh * 8, nh * 8 + 8)
        out[b_idx, sl, 0:128] = np.asarray(
            results[core]["out"]).reshape(8, 128, L).astype(np.float32)
        out[b_idx, sl, 128:256] = np.asarray(
            results[core]["out2"]).reshape(128, L).astype(np.float32)[None]
    return out.reshape(B * N, 256, L)


def kernel(**inputs):
    np_inputs = {k: np.asarray(v) for k, v in inputs.items()}
    in_maps = _host_prep(**{k: (np_inputs[k].astype(np.float32)
                                if k != "edge_index" else np_inputs[k])
                            for k in np_inputs})
    res = _run(in_maps, trace=False)
    return _assemble(res.results)


def kernel_traced(**inputs):
    """Returns (output, BassKernelResults) with NTFF profiling if available."""
    np_inputs = {k: np.asarray(v) for k, v in inputs.items()}
    in_maps = _host_prep(**{k: (np_inputs[k].astype(np.float32)
                                if k != "edge_index" else np_inputs[k])
                            for k in np_inputs})
    res = _run(in_maps, trace=True)
    return _assemble(res.results), res
